# revision 5
# baseline (speedup 1.0000x reference)
"""Multi-head self-attention (no mask) on 8 TRN2 NeuronCores.

Sharding: tensor-parallel over heads (2 heads/core) for QKV + attention,
per-(head,batch) AllToAlls re-shard to row-parallel for the output
projection.

v2: software-pipelined schedule that keeps the PE saturated end-to-end.
The PE row budget (~800k rows at the sustained-power clock) is the
floor; ACT (exp), DVE (softmax sums / normalize / biases), GPSIMD
(casts) and the collectives must all hide under it.

  stage 1: QKV for batch 0; weight + x casts fill ACT/DVE/GPSIMD.
  stage 2: attention(b0) chunk pipeline, paced by QKV(b1) filler.
  stage 3: attention(b1) pipeline, paced by out-projection(b0) filler
           and Wo second-half casts (GPSIMD).
  stage 4: leftover out-projection(b0) hides the last AllToAll, then
           out-projection(b1).

Attention chunk pipeline (1 step per chunk, chunk state lags 1-2
steps): step i emits [scores_k(i) + exp_k(i) + PV_k(i-1) + filler]
per k so the PE stream never outruns ACT's exp throughput; softmax
denominators via a single strided DVE add-reduce over expT, fast
1-pass reciprocal, PE ones-broadcast, one DVE multiply.
"""

import numpy as np

import concourse.bass as bass
import concourse.tile as tile
from concourse import bacc, mybir
from concourse.bass_utils import run_bass_kernel_spmd

F32 = mybir.dt.float32
BF16 = mybir.dt.bfloat16

B, S, H = 2, 2048, 2048
NH, HD = 16, 128
NC = 8
BS = B * S          # 4096 tokens total
FL = H // NC        # 256 features per core (2 heads)
HL = NH // NC       # 2 heads per core
K16 = H // 128      # 16 contraction tiles
CW = 512            # QKV row-chunk width (tokens per chunk)
NCHUNK = BS // CW   # 8 chunks; 0-3 = batch 0, 4-7 = batch 1
QC = 512            # attention q-chunk width
RB = S // NC        # 256 rows per core per batch after a2a
SCALE = 1.0 / float(np.sqrt(HD))

_CACHED = None


def _build():
    nc = bacc.Bacc("TRN2", target_bir_lowering=False, debug=False, num_devices=NC)

    xT_d = nc.dram_tensor("xT", [H, BS], F32, kind="ExternalInput")
    wqT_d = nc.dram_tensor("wqT", [H, FL], F32, kind="ExternalInput")
    wkT_d = nc.dram_tensor("wkT", [H, FL], F32, kind="ExternalInput")
    wvT_d = nc.dram_tensor("wvT", [H, FL], F32, kind="ExternalInput")
    bq_d = nc.dram_tensor("bq", [128, HL], F32, kind="ExternalInput")
    bk_d = nc.dram_tensor("bk", [128, HL], F32, kind="ExternalInput")
    bv_d = nc.dram_tensor("bv_bc2", [128, 2 * FL], F32, kind="ExternalInput")
    woT_d = nc.dram_tensor("woT", [H, H], F32, kind="ExternalInput")
    bo_d = nc.dram_tensor("bo_bc", [128, H], F32, kind="ExternalInput")
    onesb_d = nc.dram_tensor("ones_bf", [128, 128], BF16, kind="ExternalInput")
    out_d = nc.dram_tensor("out", [2 * RB, H], F32, kind="ExternalOutput")

    with tile.TileContext(nc) as tc:
        with (
            tc.tile_pool(name="consts", bufs=1) as cstp,
            tc.tile_pool(name="dram", bufs=1, space="DRAM") as dp,
            tc.tile_pool(name="stg", bufs=1) as stgp,
            tc.tile_pool(name="qkvout", bufs=1) as qkvp,
            tc.tile_pool(name="wo1", bufs=1) as wo1p,
            tc.tile_pool(name="attn", bufs=1) as atnp,
            tc.tile_pool(name="psum", bufs=1, space="PSUM") as pp,
        ):
            ones_bf = cstp.tile([128, 128], BF16)
            nc.sync.dma_start(ones_bf[:], onesb_d.ap()[:])
            bq_sb = cstp.tile([128, HL], F32)
            nc.sync.dma_start(bq_sb[:], bq_d.ap()[:])
            bk_sb = cstp.tile([128, HL], F32)
            nc.sync.dma_start(bk_sb[:], bk_d.ap()[:])
            bv_sb = cstp.tile([128, 2 * FL], F32)
            nc.sync.dma_start(bv_sb[:], bv_d.ap()[:])
            bo_sb = cstp.tile([128, H], F32)
            nc.sync.dma_start(bo_sb[:], bo_d.ap()[:])

            a2a_in = [[dp.tile([NC, 128, RB], BF16, name=f"a2ai{h}{b}")
                       for b in range(B)] for h in range(HL)]
            a2a_out = [[dp.tile([NC, 128, RB], BF16, name=f"a2ao{h}{b}")
                        for b in range(B)] for h in range(HL)]

            qT_sb = qkvp.tile([128, HL * BS], BF16)
            kT_sb = qkvp.tile([128, HL * BS], BF16)
            v_sb = qkvp.tile([128, (BS // 128) * FL], BF16)

            # Wo output-feature halves, k-major: wo1 = out cols 0:1024
            wo1_sb = wo1p.tile([128, K16 * 1024], BF16)

            _ceng = [0]

            def cast_load(dst_slice, src_ap, width, eng=None):
                """DMA f32 -> staging, cast to bf16 on the given engine."""
                src = stgp.tile([128, CW], F32, tag="stg", bufs=4)
                nc.sync.dma_start(src[:, :width], src_ap)
                if eng is None:
                    eng = (nc.vector, nc.scalar)[_ceng[0] % 2]
                    _ceng[0] += 1
                if eng is nc.scalar:
                    eng.activation(dst_slice, src[:, :width],
                                   mybir.ActivationFunctionType.Copy)
                else:
                    eng.tensor_copy(dst_slice, src[:, :width])

            # ---------------- QKV emitters ----------------
            xbf = {}

            def gen_qkv_qk(c):
                """Generator: q/k matmuls of chunk c; yields rows per mm."""
                xc = xbf[c]
                for w_sb, b_sb, dst in ((wq_sb, bq_sb, qT_sb),
                                        (wk_sb, bk_sb, kT_sb)):
                    for m in range(HL):
                        ps = pp.tile([128, CW], F32, tag="qk", bufs=2)
                        for k in range(K16):
                            nc.tensor.matmul(
                                ps[:],
                                w_sb[:, k * FL + m * 128: k * FL + (m + 1) * 128],
                                xc[:, k * CW:(k + 1) * CW],
                                start=(k == 0), stop=(k == K16 - 1),
                            )
                            yield CW
                        nc.vector.tensor_scalar_add(
                            dst[:, m * BS + c * CW: m * BS + (c + 1) * CW],
                            ps[:], b_sb[:, m:m + 1],
                        )

            def gen_qkv_v(c):
                xc = xbf[c]
                for m2 in range(0, CW // 128, 2):
                    ps = pp.tile([128, 2 * FL], F32, tag="vv", bufs=1)
                    for half in range(2):
                        for k in range(K16):
                            nc.tensor.matmul(
                                ps[:, half * FL:(half + 1) * FL],
                                xc[:, k * CW + (m2 + half) * 128:
                                   k * CW + (m2 + half + 1) * 128],
                                wv_sb[:, k * FL:(k + 1) * FL],
                                start=(k == 0), stop=(k == K16 - 1),
                            )
                            yield FL
                    i = c * (CW // 128) + m2
                    nc.vector.tensor_add(
                        v_sb[:, i * FL:(i + 2) * FL], ps[:], bv_sb[:]
                    )

            # ---------------- out-projection emitters ----------------
            def gen_proj(b, m, npair, aT, wo_sb):
                """rows [b*RB+m*128, +128), out cols npair*1024..+1024."""
                acc = [pp.tile([128, 512], F32, tag="qk", bufs=2,
                               name="acc") for _ in range(2)]
                for k in range(K16):
                    for j in range(2):
                        nc.tensor.matmul(
                            acc[j][:],
                            aT[:, k * RB + m * 128: k * RB + (m + 1) * 128],
                            wo_sb[:, k * 1024 + j * 512:
                                  k * 1024 + (j + 1) * 512],
                            start=(k == 0), stop=(k == K16 - 1),
                        )
                        yield 512
                for j in range(2):
                    n = npair * 2 + j
                    ot = atnp.tile([128, 512], F32, tag="ot", bufs=2)
                    nc.vector.tensor_add(
                        ot[:], acc[j][:], bo_sb[:, n * 512:(n + 1) * 512]
                    )
                    nc.sync.dma_start(
                        out_d.ap()[b * RB + m * 128: b * RB + (m + 1) * 128,
                                   n * 512:(n + 1) * 512],
                        ot[:],
                    )

            def gather_aT(b, aT, heads):
                for g in heads:
                    nc.sync.dma_start(
                        aT[:, g * RB:(g + 1) * RB],
                        a2a_out[g % 2][b][g // 2, :, :],
                    )

            def emit_a2a(h, b):
                nc.gpsimd.collective_compute(
                    "AllToAll",
                    mybir.AluOpType.bypass,
                    ins=[a2a_in[h][b].opt()],
                    outs=[a2a_out[h][b].opt()],
                    replica_groups=[list(range(NC))],
                )

            # ---------------- attention pipeline ----------------
            CH = [(h, b, qc) for b in range(B) for h in range(HL)
                  for qc in range(4)]
            pend = {}

            def emit_sc_exp(i, k):
                h, b, qc = CH[i]
                base = h * BS + b * S
                st = pend[i]
                sc = pp.tile([128, QC], F32, tag="sc", bufs=2, name="sc")
                nc.tensor.matmul(
                    sc[:],
                    kT_sb[:, base + k * 128: base + (k + 1) * 128],
                    qT_sb[:, base + qc * QC: base + (qc + 1) * QC],
                    start=True, stop=True,
                )
                nc.scalar.activation(
                    st["expT"][:, k * QC:(k + 1) * QC], sc[:],
                    mybir.ActivationFunctionType.Exp, scale=SCALE,
                )

            def emit_pv_k(i, k):
                h, b, qc = CH[i]
                st = pend[i]
                if k == 0:
                    st["pv"] = pp.tile([128, QC], F32, tag="pv", bufs=2,
                                       name="pv")
                nc.tensor.matmul(
                    st["pv"][:],
                    v_sb[:, (16 * b + k) * FL + h * 128:
                         (16 * b + k) * FL + (h + 1) * 128],
                    st["expT"][:, k * QC:(k + 1) * QC],
                    start=(k == 0), stop=(k == K16 - 1),
                )

            def emit_red(i):
                """Softmax denominator part 1: free-axis reduce + cast."""
                st = pend[i]
                red = atnp.tile([128, QC], F32, tag="red", bufs=2)
                nc.vector.tensor_reduce(
                    red[:],
                    st["expT"][:].rearrange("p (k q) -> p q k", k=K16),
                    axis=mybir.AxisListType.X, op=mybir.AluOpType.add,
                )
                s4b = atnp.tile([128, QC], BF16, tag="s4b", bufs=2)
                nc.vector.tensor_copy(s4b[:], red[:])
                st["s4b"] = s4b

            def emit_sum_recip(i):
                st = pend[i]
                pssum = pp.tile([128, QC], F32, tag="ps1", bufs=1,
                                name="pssum")
                nc.tensor.matmul(pssum[:1, :], ones_bf[:, :1], st["s4b"][:],
                                 start=True, stop=True)
                rcp = atnp.tile([1, QC], F32, tag="rcp", bufs=1)
                nc.vector.reciprocal_approx_fast(rcp[:1, :], pssum[:1, :])
                rcpb = atnp.tile([1, QC], BF16, tag="rcpb", bufs=1)
                nc.gpsimd.tensor_copy(rcpb[:1, :], rcp[:1, :])
                st["rcp"] = rcpb

            def emit_tail(i):
                """Broadcast 1/sum, normalize, scatter to a2a_in."""
                h, b, qc = CH[i]
                st = pend.pop(i)
                psb = pp.tile([128, QC], F32, tag="ps1", bufs=1, name="psb")
                nc.tensor.matmul(psb[:], ones_bf[:1, :], st["rcp"][:1, :],
                                 start=True, stop=True)
                rb = atnp.tile([128, QC], BF16, tag="rb", bufs=2)
                nc.vector.tensor_copy(rb[:], psb[:])
                att = atnp.tile([128, QC], BF16, tag="att", bufs=2)
                nc.vector.tensor_mul(att[:], st["pv"][:], rb[:])
                d0 = qc * 2
                nc.gpsimd.dma_start(a2a_in[h][b][d0, :, :], att[:, :RB])
                nc.gpsimd.dma_start(a2a_in[h][b][d0 + 1, :, :], att[:, RB:])
                if qc == 3:
                    emit_a2a(h, b)

            fillers = []

            def pull(rows_target):
                got = 0
                while got < rows_target and fillers:
                    try:
                        got += next(fillers[0])
                    except StopIteration:
                        fillers.pop(0)
                return got

            def drain_all():
                while fillers:
                    for _ in fillers.pop(0):
                        pass

            def attn_step(i, rows_budget):
                """Emit scores/exp of chunk i, PV of i-1, tail of i-2."""
                cur = i < len(CH)
                if i - 2 >= 0:
                    emit_tail(i - 2)
                if cur:
                    pend[i] = {"expT": atnp.tile([128, K16 * QC], BF16,
                                                 tag="expT", bufs=2,
                                                 name="expT")}
                if 0 <= i - 1 < len(CH):
                    emit_red(i - 1)
                per_k = max(rows_budget // K16, 384 if cur else 0)
                for k in range(K16):
                    if cur:
                        emit_sc_exp(i, k)
                    if 0 <= i - 1 < len(CH):
                        emit_pv_k(i - 1, k)
                    pull(per_k)
                if 0 <= i - 1 < len(CH):
                    emit_sum_recip(i - 1)

            # ================= stage 1: QKV batch 0 =================
            with (
                tc.tile_pool(name="wqkv", bufs=1) as wp,
                tc.tile_pool(name="xwin", bufs=1) as xbp,
            ):
                def load_w(dram, tag):
                    dst = wp.tile([128, K16 * FL], BF16, tag=tag, name=tag)
                    for k in range(K16):
                        cast_load(dst[:, k * FL:(k + 1) * FL],
                                  dram.ap()[k * 128:(k + 1) * 128, :], FL)
                    return dst

                def load_x(c, eng):
                    dst = xbp.tile([128, K16 * CW], BF16, tag="xbf", bufs=2,
                                   name=f"xbf{c}")
                    for k in range(K16):
                        cast_load(dst[:, k * CW:(k + 1) * CW],
                                  xT_d.ap()[k * 128:(k + 1) * 128,
                                            c * CW:(c + 1) * CW], CW, eng)
                    xbf[c] = dst

                wq_sb = load_w(wqT_d, "w_q")
                load_x(0, nc.gpsimd)
                wk_sb = load_w(wkT_d, "w_k")
                wv_sb = load_w(wvT_d, "w_v")

                for c in range(4):
                    load_x(c + 1, nc.gpsimd)
                    for _ in gen_qkv_qk(c):
                        pass
                    for _ in gen_qkv_v(c):
                        pass
                    # wo1 casts ride the stage-1 slack on ACT/DVE
                    for k in range(4 * c, 4 * (c + 1)):
                        cast_load(wo1_sb[:, k * 1024: k * 1024 + 512],
                                  woT_d.ap()[k * 128:(k + 1) * 128, 0:512],
                                  512)
                        cast_load(wo1_sb[:, k * 1024 + 512:(k + 1) * 1024],
                                  woT_d.ap()[k * 128:(k + 1) * 128, 512:1024],
                                  512)

                # ===== stage 2: attention b0 paced by QKV b1 =====
                def gen_qkv_b1():
                    for c in range(4, NCHUNK):
                        if c + 1 < NCHUNK:
                            load_x(c + 1, nc.gpsimd)
                        yield from gen_qkv_qk(c)
                        yield from gen_qkv_v(c)

                fillers.append(gen_qkv_b1())
                for i in range(8):
                    attn_step(i, 23000 if i < 7 else 8000)
                # all b1 QKV must be emitted before b1 scores read it
                drain_all()

            # ===== stage 3: attention b1 paced by out-projection b0 =====
            with (
                tc.tile_pool(name="wo2", bufs=1) as wo2p,
                tc.tile_pool(name="aTp", bufs=1) as atp,
            ):
                wo2_sb = wo2p.tile([128, K16 * 1024], BF16)
                aT0 = atp.tile([128, K16 * RB], BF16, name="aT0")
                aT1 = atp.tile([128, K16 * RB], BF16, name="aT1")

                def gen_wo2_casts():
                    for k in range(K16):
                        cast_load(wo2_sb[:, k * 1024: k * 1024 + 512],
                                  woT_d.ap()[k * 128:(k + 1) * 128,
                                             1024:1536], 512, nc.gpsimd)
                        cast_load(wo2_sb[:, k * 1024 + 512:(k + 1) * 1024],
                                  woT_d.ap()[k * 128:(k + 1) * 128,
                                             1536:2048], 512, nc.gpsimd)
                        yield 600

                def gen_proj_b0():
                    gather_aT(0, aT0, range(16))
                    yield 0
                    for m, npair in ((0, 0), (0, 1), (1, 0)):
                        yield from gen_proj(0, m, npair, aT0,
                                            wo1_sb if npair == 0 else wo2_sb)

                for i in range(8, 16):
                    attn_step(i, 9000 if i >= 10 else 0)
                    if i == 9:
                        # a2a(1,0) was emitted in this step's tail
                        fillers.append(gen_wo2_casts())
                        fillers.append(gen_proj_b0())
                # drain steps: finish chunks 14/15
                attn_step(16, 6000)
                attn_step(17, 6000)
                drain_all()

                # ===== stage 4: out-projection b1 =====
                gather_aT(1, aT1, range(16))
                for _ in gen_proj(0, 1, 1, aT0, wo2_sb):
                    pass
                for m in range(2):
                    for npair in range(2):
                        for _ in gen_proj(1, m, npair, aT1,
                                          wo1_sb if npair == 0 else wo2_sb):
                            pass

    nc.compile()
    return nc


def _get_nc():
    global _CACHED
    if _CACHED is None:
        _CACHED = _build()
    return _CACHED


def _prep_in_maps(x, Wq, bq, Wk, bk, Wv, bv, Wo, bo):
    import ml_dtypes

    xT = np.ascontiguousarray(x.reshape(BS, H).T)
    woT = np.ascontiguousarray(Wo.T)
    bo_bc = np.ascontiguousarray(np.broadcast_to(bo, (128, H)))
    ones_bf = np.ones((128, 128), ml_dtypes.bfloat16)
    in_maps = []
    for c in range(NC):
        sl = slice(FL * c, FL * (c + 1))
        in_maps.append(
            {
                "xT": xT,
                "wqT": np.ascontiguousarray(Wq[sl, :].T),
                "wkT": np.ascontiguousarray(Wk[sl, :].T),
                "wvT": np.ascontiguousarray(Wv[sl, :].T),
                "bq": np.ascontiguousarray(bq[sl].reshape(HL, 128).T),
                "bk": np.ascontiguousarray(bk[sl].reshape(HL, 128).T),
                "bv_bc2": np.ascontiguousarray(
                    np.broadcast_to(np.tile(bv[sl], 2), (128, 2 * FL))),
                "woT": woT,
                "bo_bc": bo_bc,
                "ones_bf": ones_bf,
            }
        )
    return in_maps


def run(in_maps, trace=False):
    nc = _get_nc()
    return run_bass_kernel_spmd(nc, in_maps, core_ids=list(range(NC)), trace=trace)


def kernel(x, Wq, bq, Wk, bk, Wv, bv, Wo, bo):
    args = [np.asarray(a, dtype=np.float32)
            for a in (x, Wq, bq, Wk, bk, Wv, bv, Wo, bo)]
    in_maps = _prep_in_maps(*args)
    res = run(in_maps)
    out = np.empty((B, S, H), dtype=np.float32)
    for c in range(NC):
        oc = res.results[c]["out"]
        out[0, c * RB:(c + 1) * RB] = oc[:RB]
        out[1, c * RB:(c + 1) * RB] = oc[RB:]
    return out


# revision 10
# speedup vs baseline: 1.2694x; 1.2694x over previous
"""Multi-head self-attention (no mask) on 8 TRN2 NeuronCores.

Sharding: tensor-parallel over heads (2 heads/core) for QKV + attention,
per-(head,batch) AllToAlls re-shard to row-parallel for the output
projection.

v2: software-pipelined schedule that keeps the PE saturated end-to-end.
The PE row budget (~800k rows at the sustained-power clock) is the
floor; ACT (exp), DVE (softmax sums / normalize / biases), GPSIMD
(casts) and the collectives must all hide under it.

  stage 1: QKV for batch 0; weight + x casts fill ACT/DVE/GPSIMD.
  stage 2: attention(b0) chunk pipeline, paced by QKV(b1) filler.
  stage 3: attention(b1) pipeline, paced by out-projection(b0) filler
           and Wo second-half casts (GPSIMD).
  stage 4: leftover out-projection(b0) hides the last AllToAll, then
           out-projection(b1).

Attention chunk pipeline (1 step per chunk, chunk state lags 1-2
steps): step i emits [scores_k(i) + exp_k(i) + PV_k(i-1) + filler]
per k so the PE stream never outruns ACT's exp throughput; softmax
denominators via a single strided DVE add-reduce over expT, fast
1-pass reciprocal, PE ones-broadcast, one DVE multiply.
"""

import numpy as np

import concourse.bass as bass
import concourse.tile as tile
from concourse import bacc, mybir
from concourse.bass_utils import run_bass_kernel_spmd

F32 = mybir.dt.float32
BF16 = mybir.dt.bfloat16

B, S, H = 2, 2048, 2048
NH, HD = 16, 128
NC = 8
BS = B * S          # 4096 tokens total
FL = H // NC        # 256 features per core (2 heads)
HL = NH // NC       # 2 heads per core
K16 = H // 128      # 16 contraction tiles
CW = 512            # QKV row-chunk width (tokens per chunk)
NCHUNK = BS // CW   # 8 chunks; 0-3 = batch 0, 4-7 = batch 1
QC = 512            # attention q-chunk width
RB = S // NC        # 256 rows per core per batch after a2a
SCALE = 1.0 / float(np.sqrt(HD))

_CACHED = None


def _build():
    nc = bacc.Bacc("TRN2", target_bir_lowering=False, debug=False, num_devices=NC)

    xT_d = nc.dram_tensor("xT", [H, BS], F32, kind="ExternalInput")
    wqT_d = nc.dram_tensor("wqT", [H, FL], F32, kind="ExternalInput")
    wkT_d = nc.dram_tensor("wkT", [H, FL], F32, kind="ExternalInput")
    wvT_d = nc.dram_tensor("wvT", [H, FL], F32, kind="ExternalInput")
    bq_d = nc.dram_tensor("bq", [128, HL], F32, kind="ExternalInput")
    bk_d = nc.dram_tensor("bk", [128, HL], F32, kind="ExternalInput")
    bv_d = nc.dram_tensor("bv_bc2", [128, 2 * FL], F32, kind="ExternalInput")
    woT_d = nc.dram_tensor("woT", [H, H], F32, kind="ExternalInput")
    bo_d = nc.dram_tensor("bo_bc", [128, H], F32, kind="ExternalInput")
    onesb_d = nc.dram_tensor("ones_bf", [128, 128], BF16, kind="ExternalInput")
    out_d = nc.dram_tensor("out", [2 * RB, H], F32, kind="ExternalOutput")

    with tile.TileContext(nc) as tc:
        with (
            tc.tile_pool(name="consts", bufs=1) as cstp,
            tc.tile_pool(name="dram", bufs=1, space="DRAM") as dp,
            tc.tile_pool(name="stg", bufs=1) as stgp,
            tc.tile_pool(name="qkvout", bufs=1) as qkvp,
            tc.tile_pool(name="wo1", bufs=1) as wo1p,
            tc.tile_pool(name="attn", bufs=1) as atnp,
            tc.tile_pool(name="psum", bufs=1, space="PSUM") as pp,
        ):
            ones_bf = cstp.tile([128, 128], BF16)
            nc.sync.dma_start(ones_bf[:], onesb_d.ap()[:])
            bq_sb = cstp.tile([128, HL], F32)
            nc.sync.dma_start(bq_sb[:], bq_d.ap()[:])
            bk_sb = cstp.tile([128, HL], F32)
            nc.sync.dma_start(bk_sb[:], bk_d.ap()[:])
            bv_sb = cstp.tile([128, 2 * FL], F32)
            nc.sync.dma_start(bv_sb[:], bv_d.ap()[:])
            bo_sb = cstp.tile([128, H], F32)
            nc.sync.dma_start(bo_sb[:], bo_d.ap()[:])

            a2a_in = [[dp.tile([NC, 128, RB], BF16, name=f"a2ai{h}{b}")
                       for b in range(B)] for h in range(HL)]
            a2a_out = [[dp.tile([NC, 128, RB], BF16, name=f"a2ao{h}{b}")
                        for b in range(B)] for h in range(HL)]

            qT_sb = qkvp.tile([128, HL * BS], BF16)
            kT_sb = qkvp.tile([128, HL * BS], BF16)
            v_sb = qkvp.tile([128, (BS // 128) * FL], BF16)

            # Wo output-feature halves, k-major: wo1 = out cols 0:1024
            wo1_sb = wo1p.tile([128, K16 * 1024], BF16)

            _ceng = [0]

            def cast_load(dst_slice, src_ap, width, eng=None):
                """DMA f32 -> staging, cast to bf16 on the given engine."""
                src = stgp.tile([128, CW], F32, tag="stg", bufs=4)
                nc.sync.dma_start(src[:, :width], src_ap)
                if eng is None:
                    eng = (nc.vector, nc.scalar)[_ceng[0] % 2]
                    _ceng[0] += 1
                if eng is nc.scalar:
                    eng.activation(dst_slice, src[:, :width],
                                   mybir.ActivationFunctionType.Copy)
                else:
                    eng.tensor_copy(dst_slice, src[:, :width])

            # ---------------- QKV emitters ----------------
            xbf = {}

            def gen_qkv_qk(c):
                """Generator: q/k matmuls of chunk c; yields rows per mm."""
                xc = xbf[c]
                for w_sb, b_sb, dst in ((wq_sb, bq_sb, qT_sb),
                                        (wk_sb, bk_sb, kT_sb)):
                    for m in range(HL):
                        ps = pp.tile([128, CW], F32, tag="qk", bufs=2)
                        for k in range(K16):
                            nc.tensor.matmul(
                                ps[:],
                                w_sb[:, k * FL + m * 128: k * FL + (m + 1) * 128],
                                xc[:, k * CW:(k + 1) * CW],
                                start=(k == 0), stop=(k == K16 - 1),
                            )
                            yield CW
                        nc.vector.tensor_scalar_add(
                            dst[:, m * BS + c * CW: m * BS + (c + 1) * CW],
                            ps[:], b_sb[:, m:m + 1],
                        )

            def gen_qkv_v(c):
                xc = xbf[c]
                for m2 in range(0, CW // 128, 2):
                    ps = pp.tile([128, 2 * FL], F32, tag="vv", bufs=1)
                    for half in range(2):
                        for k in range(K16):
                            nc.tensor.matmul(
                                ps[:, half * FL:(half + 1) * FL],
                                xc[:, k * CW + (m2 + half) * 128:
                                   k * CW + (m2 + half + 1) * 128],
                                wv_sb[:, k * FL:(k + 1) * FL],
                                start=(k == 0), stop=(k == K16 - 1),
                            )
                            yield FL
                    i = c * (CW // 128) + m2
                    nc.vector.tensor_add(
                        v_sb[:, i * FL:(i + 2) * FL], ps[:], bv_sb[:]
                    )

            # ---------------- out-projection emitters ----------------
            def gen_proj(b, m, npair, aT, wo_sb):
                """rows [b*RB+m*128, +128), out cols npair*1024..+1024."""
                acc = [pp.tile([128, 512], F32, tag="qk", bufs=2,
                               name="acc") for _ in range(2)]
                for k in range(K16):
                    for j in range(2):
                        nc.tensor.matmul(
                            acc[j][:],
                            aT[:, k * RB + m * 128: k * RB + (m + 1) * 128],
                            wo_sb[:, k * 1024 + j * 512:
                                  k * 1024 + (j + 1) * 512],
                            start=(k == 0), stop=(k == K16 - 1),
                        )
                        yield 512
                for j in range(2):
                    n = npair * 2 + j
                    ot = atnp.tile([128, 512], F32, tag="ot", bufs=2)
                    nc.vector.tensor_add(
                        ot[:], acc[j][:], bo_sb[:, n * 512:(n + 1) * 512]
                    )
                    nc.sync.dma_start(
                        out_d.ap()[b * RB + m * 128: b * RB + (m + 1) * 128,
                                   n * 512:(n + 1) * 512],
                        ot[:],
                    )

            def gather_aT(b, aT, heads):
                for g in heads:
                    nc.sync.dma_start(
                        aT[:, g * RB:(g + 1) * RB],
                        a2a_out[g % 2][b][g // 2, :, :],
                    )

            def emit_a2a(h, b):
                nc.gpsimd.collective_compute(
                    "AllToAll",
                    mybir.AluOpType.bypass,
                    ins=[a2a_in[h][b].opt()],
                    outs=[a2a_out[h][b].opt()],
                    replica_groups=[list(range(NC))],
                )

            # ---------------- attention pipeline ----------------
            CH = [(h, b, qc) for b in range(B) for h in range(HL)
                  for qc in range(4)]
            pend = {}

            def emit_sc_exp(i, k):
                h, b, qc = CH[i]
                base = h * BS + b * S
                st = pend[i]
                sc = pp.tile([128, QC], F32, tag="sc", bufs=2, name="sc")
                nc.tensor.matmul(
                    sc[:],
                    kT_sb[:, base + k * 128: base + (k + 1) * 128],
                    qT_sb[:, base + qc * QC: base + (qc + 1) * QC],
                    start=True, stop=True,
                )
                nc.scalar.activation(
                    st["expT"][:, k * QC:(k + 1) * QC], sc[:],
                    mybir.ActivationFunctionType.Exp, scale=SCALE,
                )

            def emit_pv_k(i, k):
                h, b, qc = CH[i]
                st = pend[i]
                if k == 0:
                    st["pv"] = pp.tile([128, QC], F32, tag="pv", bufs=2,
                                       name="pv")
                nc.tensor.matmul(
                    st["pv"][:],
                    v_sb[:, (16 * b + k) * FL + h * 128:
                         (16 * b + k) * FL + (h + 1) * 128],
                    st["expT"][:, k * QC:(k + 1) * QC],
                    start=(k == 0), stop=(k == K16 - 1),
                )

            def emit_sum_recip(i):
                """Denominator: in-place pairwise tree over expT (safe:
                PV of chunk i has already consumed it), ones-matmul for
                the partition sum, fast reciprocal."""
                st = pend[i]
                e = st["expT"]
                nc.vector.tensor_add(e[:, :8 * QC], e[:, :8 * QC],
                                     e[:, 8 * QC:])
                nc.vector.tensor_add(e[:, :4 * QC], e[:, :4 * QC],
                                     e[:, 4 * QC:8 * QC])
                nc.vector.tensor_add(e[:, :2 * QC], e[:, :2 * QC],
                                     e[:, 2 * QC:4 * QC])
                nc.vector.tensor_add(e[:, :QC], e[:, :QC], e[:, QC:2 * QC])
                pssum = pp.tile([128, QC], F32, tag="ps1", bufs=1,
                                name="pssum")
                nc.tensor.matmul(pssum[:1, :], ones_bf[:, :1], e[:, :QC],
                                 start=True, stop=True)
                rcp = atnp.tile([1, QC], F32, tag="rcp", bufs=1)
                nc.vector.reciprocal_approx_fast(rcp[:1, :], pssum[:1, :])
                rcpb = atnp.tile([1, QC], BF16, tag="rcpb", bufs=1)
                nc.vector.tensor_copy(rcpb[:1, :], rcp[:1, :])
                st["rcp"] = rcpb

            def emit_tail(i):
                """Broadcast 1/sum, normalize, scatter to a2a_in."""
                h, b, qc = CH[i]
                st = pend.pop(i)
                psb = pp.tile([128, QC], F32, tag="ps1", bufs=1, name="psb")
                nc.tensor.matmul(psb[:], ones_bf[:1, :], st["rcp"][:1, :],
                                 start=True, stop=True)
                rb = atnp.tile([128, QC], BF16, tag="rb", bufs=2)
                nc.vector.tensor_copy(rb[:], psb[:])
                att = atnp.tile([128, QC], BF16, tag="att", bufs=2)
                nc.vector.tensor_mul(att[:], st["pv"][:], rb[:])
                d0 = qc * 2
                nc.gpsimd.dma_start(a2a_in[h][b][d0, :, :], att[:, :RB])
                nc.gpsimd.dma_start(a2a_in[h][b][d0 + 1, :, :], att[:, RB:])
                if qc == 3:
                    emit_a2a(h, b)

            fillers = []

            def pull(rows_target):
                got = 0
                while got < rows_target and fillers:
                    try:
                        got += next(fillers[0])
                    except StopIteration:
                        fillers.pop(0)
                return got

            def drain_all():
                while fillers:
                    for _ in fillers.pop(0):
                        pass

            def attn_step(i, rows_budget):
                """Emit scores/exp of chunk i, PV of i-1, tail of i-2."""
                cur = i < len(CH)
                if i - 2 >= 0:
                    emit_tail(i - 2)
                if cur:
                    pend[i] = {"expT": atnp.tile([128, K16 * QC], BF16,
                                                 tag="expT", bufs=2,
                                                 name="expT")}
                per_k = max(rows_budget // K16, 384 if cur else 0)
                for k in range(K16):
                    if cur:
                        emit_sc_exp(i, k)
                    if 0 <= i - 1 < len(CH):
                        emit_pv_k(i - 1, k)
                    pull(per_k)
                if 0 <= i - 1 < len(CH):
                    emit_sum_recip(i - 1)

            # ================= stage 1: QKV batch 0 =================
            with (
                tc.tile_pool(name="wqkv", bufs=1) as wp,
                tc.tile_pool(name="xwin", bufs=1) as xbp,
            ):
                def load_w(dram, tag):
                    dst = wp.tile([128, K16 * FL], BF16, tag=tag, name=tag)
                    for k in range(K16):
                        cast_load(dst[:, k * FL:(k + 1) * FL],
                                  dram.ap()[k * 128:(k + 1) * 128, :], FL)
                    return dst

                def load_x(c):
                    dst = xbp.tile([128, K16 * CW], BF16, tag="xbf", bufs=2,
                                   name=f"xbf{c}")
                    for k in range(K16):
                        cast_load(dst[:, k * CW:(k + 1) * CW],
                                  xT_d.ap()[k * 128:(k + 1) * 128,
                                            c * CW:(c + 1) * CW], CW)
                    xbf[c] = dst

                # interleave wq and x0 so the first q matmuls can start
                # as soon as the tail of both cast streams lands
                wq_sb = wp.tile([128, K16 * FL], BF16, tag="w_q", name="w_q")
                x0 = xbp.tile([128, K16 * CW], BF16, tag="xbf", bufs=2,
                              name="xbf0")
                for k in range(K16):
                    cast_load(wq_sb[:, k * FL:(k + 1) * FL],
                              wqT_d.ap()[k * 128:(k + 1) * 128, :], FL)
                    cast_load(x0[:, k * CW:(k + 1) * CW],
                              xT_d.ap()[k * 128:(k + 1) * 128, 0:CW], CW)
                xbf[0] = x0
                wk_sb = load_w(wkT_d, "w_k")
                wv_sb = load_w(wvT_d, "w_v")

                for c in range(4):
                    load_x(c + 1)
                    for _ in gen_qkv_qk(c):
                        pass
                    for _ in gen_qkv_v(c):
                        pass
                    # wo1 casts ride the stage-1 slack on ACT/DVE
                    for k in range(4 * c, 4 * (c + 1)):
                        cast_load(wo1_sb[:, k * 1024: k * 1024 + 512],
                                  woT_d.ap()[k * 128:(k + 1) * 128, 0:512],
                                  512)
                        cast_load(wo1_sb[:, k * 1024 + 512:(k + 1) * 1024],
                                  woT_d.ap()[k * 128:(k + 1) * 128, 512:1024],
                                  512)

                # ===== stage 2: attention b0 paced by QKV b1 =====
                def gen_qkv_b1():
                    for c in range(4, NCHUNK):
                        if c + 1 < NCHUNK:
                            load_x(c + 1)
                        yield from gen_qkv_qk(c)
                        yield from gen_qkv_v(c)

                fillers.append(gen_qkv_b1())
                for i in range(8):
                    attn_step(i, 26000 if i < 7 else 8000)
                # all b1 QKV must be emitted before b1 scores read it
                drain_all()

            # ===== stage 3: attention b1 paced by out-projection b0 =====
            with (
                tc.tile_pool(name="wo2", bufs=1) as wo2p,
                tc.tile_pool(name="aTp", bufs=1) as atp,
            ):
                wo2_sb = wo2p.tile([128, K16 * 1024], BF16)
                aT0 = atp.tile([128, K16 * RB], BF16, name="aT0")
                aT1 = atp.tile([128, K16 * RB], BF16, name="aT1")

                def gen_wo2_casts():
                    for k in range(K16):
                        cast_load(wo2_sb[:, k * 1024: k * 1024 + 512],
                                  woT_d.ap()[k * 128:(k + 1) * 128,
                                             1024:1536], 512, nc.vector)
                        cast_load(wo2_sb[:, k * 1024 + 512:(k + 1) * 1024],
                                  woT_d.ap()[k * 128:(k + 1) * 128,
                                             1536:2048], 512, nc.vector)
                        yield 600

                def gen_proj_b0():
                    gather_aT(0, aT0, range(16))
                    yield 0
                    for m in range(2):
                        yield from gen_proj(0, m, 0, aT0, wo1_sb)

                for i in range(8, 16):
                    attn_step(i, 8700 if i >= 10 else 0)
                    if i == 9:
                        # a2a(1,0) was emitted in this step's tail
                        fillers.append(gen_wo2_casts())
                        fillers.append(gen_proj_b0())
                # drain steps: finish chunks 14/15
                attn_step(16, 6000)
                attn_step(17, 6000)
                drain_all()

                # ===== stage 4: reserved b0 pieces hide the last a2a =====
                gather_aT(1, aT1, range(16))
                for m in range(2):
                    for _ in gen_proj(0, m, 1, aT0, wo2_sb):
                        pass
                for m in range(2):
                    for npair in range(2):
                        for _ in gen_proj(1, m, npair, aT1,
                                          wo1_sb if npair == 0 else wo2_sb):
                            pass

    nc.compile()
    return nc


def _get_nc():
    global _CACHED
    if _CACHED is None:
        _CACHED = _build()
    return _CACHED


def _prep_in_maps(x, Wq, bq, Wk, bk, Wv, bv, Wo, bo):
    import ml_dtypes

    xT = np.ascontiguousarray(x.reshape(BS, H).T)
    woT = np.ascontiguousarray(Wo.T)
    bo_bc = np.ascontiguousarray(np.broadcast_to(bo, (128, H)))
    ones_bf = np.ones((128, 128), ml_dtypes.bfloat16)
    in_maps = []
    for c in range(NC):
        sl = slice(FL * c, FL * (c + 1))
        in_maps.append(
            {
                "xT": xT,
                "wqT": np.ascontiguousarray(Wq[sl, :].T),
                "wkT": np.ascontiguousarray(Wk[sl, :].T),
                "wvT": np.ascontiguousarray(Wv[sl, :].T),
                "bq": np.ascontiguousarray(bq[sl].reshape(HL, 128).T),
                "bk": np.ascontiguousarray(bk[sl].reshape(HL, 128).T),
                "bv_bc2": np.ascontiguousarray(
                    np.broadcast_to(np.tile(bv[sl], 2), (128, 2 * FL))),
                "woT": woT,
                "bo_bc": bo_bc,
                "ones_bf": ones_bf,
            }
        )
    return in_maps


def run(in_maps, trace=False):
    nc = _get_nc()
    return run_bass_kernel_spmd(nc, in_maps, core_ids=list(range(NC)), trace=trace)


def kernel(x, Wq, bq, Wk, bk, Wv, bv, Wo, bo):
    args = [np.asarray(a, dtype=np.float32)
            for a in (x, Wq, bq, Wk, bk, Wv, bv, Wo, bo)]
    in_maps = _prep_in_maps(*args)
    res = run(in_maps)
    out = np.empty((B, S, H), dtype=np.float32)
    for c in range(NC):
        oc = res.results[c]["out"]
        out[0, c * RB:(c + 1) * RB] = oc[:RB]
        out[1, c * RB:(c + 1) * RB] = oc[RB:]
    return out


# revision 11
# speedup vs baseline: 1.3978x; 1.1011x over previous
"""Multi-head self-attention (no mask) on 8 TRN2 NeuronCores.

Sharding: tensor-parallel over heads (2 heads/core) for QKV + attention,
per-(head,batch) AllToAlls re-shard to row-parallel for the output
projection.

v4: software-pipelined schedule that keeps the PE saturated end-to-end.
The PE row budget (~800k rows at the sustained-power clock) is the
floor; ACT (exp), DVE (softmax sums / normalize / biases) and the
collectives must all hide under it.  All f32->bf16 input casting is
done host-side (like the layout transposes), so DMA volume is halved
and ACT/DVE do no cast work.

  stage 1: QKV for batch 0.
  stage 2: attention(b0) chunk pipeline, paced by QKV(b1) filler.
  stage 3: attention(b1) pipeline, paced by out-projection(b0) filler.
  stage 4: reserved out-projection(b0) pieces hide the last AllToAll,
           then out-projection(b1).

Attention chunk pipeline (1 step per chunk, chunk state lags 1-2
steps): step i emits [scores_k(i) + exp_k(i) + PV_k(i-1) + filler]
per k so the PE stream never outruns ACT's exp throughput (718ns per
512-col tile vs 262ns per matmul).  Softmax denominators: in-place
pairwise DVE tree over expT (interleaved into the PV k-loop), PE
ones-matmul partition sum, 1-pass fast reciprocal, PE broadcast,
one DVE multiply.
"""

import numpy as np

import concourse.bass as bass
import concourse.tile as tile
from concourse import bacc, mybir
from concourse.bass_utils import run_bass_kernel_spmd

F32 = mybir.dt.float32
BF16 = mybir.dt.bfloat16

B, S, H = 2, 2048, 2048
NH, HD = 16, 128
NC = 8
BS = B * S          # 4096 tokens total
FL = H // NC        # 256 features per core (2 heads)
HL = NH // NC       # 2 heads per core
K16 = H // 128      # 16 contraction tiles
CW = 512            # QKV row-chunk width (tokens per chunk)
NCHUNK = BS // CW   # 8 chunks; 0-3 = batch 0, 4-7 = batch 1
QC = 512            # attention q-chunk width
RB = S // NC        # 256 rows per core per batch after a2a
SCALE = 1.0 / float(np.sqrt(HD))

_CACHED = None


def _build():
    nc = bacc.Bacc("TRN2", target_bir_lowering=False, debug=False, num_devices=NC)

    xT_d = nc.dram_tensor("xTb", [H, BS], BF16, kind="ExternalInput")
    wqT_d = nc.dram_tensor("wqTb", [H, FL], BF16, kind="ExternalInput")
    wkT_d = nc.dram_tensor("wkTb", [H, FL], BF16, kind="ExternalInput")
    wvT_d = nc.dram_tensor("wvTb", [H, FL], BF16, kind="ExternalInput")
    bq_d = nc.dram_tensor("bq", [128, HL], F32, kind="ExternalInput")
    bk_d = nc.dram_tensor("bk", [128, HL], F32, kind="ExternalInput")
    bv_d = nc.dram_tensor("bv_bc2", [128, 2 * FL], F32, kind="ExternalInput")
    woT_d = nc.dram_tensor("woTb", [H, H], BF16, kind="ExternalInput")
    bo_d = nc.dram_tensor("bo_bc", [128, H], F32, kind="ExternalInput")
    onesb_d = nc.dram_tensor("ones_bf", [128, 128], BF16, kind="ExternalInput")
    out_d = nc.dram_tensor("out", [2 * RB, H], F32, kind="ExternalOutput")

    with tile.TileContext(nc) as tc:
        with (
            tc.tile_pool(name="consts", bufs=1) as cstp,
            tc.tile_pool(name="dram", bufs=1, space="DRAM") as dp,
            tc.tile_pool(name="qkvout", bufs=1) as qkvp,
            tc.tile_pool(name="wo1", bufs=1) as wo1p,
            tc.tile_pool(name="attn", bufs=1) as atnp,
            tc.tile_pool(name="psum", bufs=1, space="PSUM") as pp,
        ):
            ones_bf = cstp.tile([128, 128], BF16)
            nc.sync.dma_start(ones_bf[:], onesb_d.ap()[:])
            bq_sb = cstp.tile([128, HL], F32)
            nc.sync.dma_start(bq_sb[:], bq_d.ap()[:])
            bk_sb = cstp.tile([128, HL], F32)
            nc.sync.dma_start(bk_sb[:], bk_d.ap()[:])
            bv_sb = cstp.tile([128, 2 * FL], F32)
            nc.sync.dma_start(bv_sb[:], bv_d.ap()[:])
            bo_sb = cstp.tile([128, H], F32)
            nc.sync.dma_start(bo_sb[:], bo_d.ap()[:])

            a2a_in = [[dp.tile([NC, 128, RB], BF16, name=f"a2ai{h}{b}")
                       for b in range(B)] for h in range(HL)]
            a2a_out = [[dp.tile([NC, 128, RB], BF16, name=f"a2ao{h}{b}")
                        for b in range(B)] for h in range(HL)]

            qT_sb = qkvp.tile([128, HL * BS], BF16)
            kT_sb = qkvp.tile([128, HL * BS], BF16)
            v_sb = qkvp.tile([128, (BS // 128) * FL], BF16)

            # Wo output-feature halves, k-major: wo1 = out cols 0:1024
            wo1_sb = wo1p.tile([128, K16 * 1024], BF16)
            for k in range(K16):
                nc.sync.dma_start(wo1_sb[:, k * 1024:(k + 1) * 1024],
                                  woT_d.ap()[k * 128:(k + 1) * 128, 0:1024])

            # ---------------- QKV emitters ----------------
            xbf = {}

            def gen_qkv_qk(c):
                """Generator: q/k matmuls of chunk c; yields rows per mm."""
                xc = xbf[c]
                for w_sb, b_sb, dst in ((wq_sb, bq_sb, qT_sb),
                                        (wk_sb, bk_sb, kT_sb)):
                    for m in range(HL):
                        ps = pp.tile([128, CW], F32, tag="qk", bufs=2)
                        for k in range(K16):
                            nc.tensor.matmul(
                                ps[:],
                                w_sb[:, k * FL + m * 128: k * FL + (m + 1) * 128],
                                xc[:, k * CW:(k + 1) * CW],
                                start=(k == 0), stop=(k == K16 - 1),
                            )
                            yield CW
                        nc.vector.tensor_scalar_add(
                            dst[:, m * BS + c * CW: m * BS + (c + 1) * CW],
                            ps[:], b_sb[:, m:m + 1],
                        )

            def gen_qkv_v(c):
                xc = xbf[c]
                for m2 in range(0, CW // 128, 2):
                    ps = pp.tile([128, 2 * FL], F32, tag="vv", bufs=1)
                    for half in range(2):
                        for k in range(K16):
                            nc.tensor.matmul(
                                ps[:, half * FL:(half + 1) * FL],
                                xc[:, k * CW + (m2 + half) * 128:
                                   k * CW + (m2 + half + 1) * 128],
                                wv_sb[:, k * FL:(k + 1) * FL],
                                start=(k == 0), stop=(k == K16 - 1),
                            )
                            yield FL
                    i = c * (CW // 128) + m2
                    nc.vector.tensor_add(
                        v_sb[:, i * FL:(i + 2) * FL], ps[:], bv_sb[:]
                    )

            # ---------------- out-projection emitters ----------------
            def gen_proj(b, m, npair, aT, wo_sb):
                """rows [b*RB+m*128, +128), out cols npair*1024..+1024."""
                acc = [pp.tile([128, 512], F32, tag="qk", bufs=2,
                               name="acc") for _ in range(2)]
                for k in range(K16):
                    for j in range(2):
                        nc.tensor.matmul(
                            acc[j][:],
                            aT[:, k * RB + m * 128: k * RB + (m + 1) * 128],
                            wo_sb[:, k * 1024 + j * 512:
                                  k * 1024 + (j + 1) * 512],
                            start=(k == 0), stop=(k == K16 - 1),
                        )
                        yield 512
                for j in range(2):
                    n = npair * 2 + j
                    ot = atnp.tile([128, 512], F32, tag="ot", bufs=2)
                    nc.vector.tensor_add(
                        ot[:], acc[j][:], bo_sb[:, n * 512:(n + 1) * 512]
                    )
                    nc.sync.dma_start(
                        out_d.ap()[b * RB + m * 128: b * RB + (m + 1) * 128,
                                   n * 512:(n + 1) * 512],
                        ot[:],
                    )

            def gather_aT(b, aT, heads):
                for g in heads:
                    nc.sync.dma_start(
                        aT[:, g * RB:(g + 1) * RB],
                        a2a_out[g % 2][b][g // 2, :, :],
                    )

            def emit_a2a(h, b):
                nc.gpsimd.collective_compute(
                    "AllToAll",
                    mybir.AluOpType.bypass,
                    ins=[a2a_in[h][b].opt()],
                    outs=[a2a_out[h][b].opt()],
                    replica_groups=[list(range(NC))],
                )

            # ---------------- attention pipeline ----------------
            CH = [(h, b, qc) for b in range(B) for h in range(HL)
                  for qc in range(4)]
            pend = {}

            def emit_sc_exp(i, k):
                h, b, qc = CH[i]
                base = h * BS + b * S
                st = pend[i]
                sc = pp.tile([128, QC], F32, tag="sc", bufs=2, name="sc")
                nc.tensor.matmul(
                    sc[:],
                    kT_sb[:, base + k * 128: base + (k + 1) * 128],
                    qT_sb[:, base + qc * QC: base + (qc + 1) * QC],
                    start=True, stop=True,
                )
                nc.scalar.activation(
                    st["expT"][:, k * QC:(k + 1) * QC], sc[:],
                    mybir.ActivationFunctionType.Exp, scale=SCALE,
                )

            def emit_pv_k(i, k):
                h, b, qc = CH[i]
                st = pend[i]
                if k == 0:
                    st["pv"] = pp.tile([128, QC], F32, tag="pv", bufs=2,
                                       name="pv")
                nc.tensor.matmul(
                    st["pv"][:],
                    v_sb[:, (16 * b + k) * FL + h * 128:
                         (16 * b + k) * FL + (h + 1) * 128],
                    st["expT"][:, k * QC:(k + 1) * QC],
                    start=(k == 0), stop=(k == K16 - 1),
                )

            def emit_tree(i, level):
                """In-place pairwise tree level over expT (PV has already
                consumed the halves being overwritten)."""
                e = pend[i]["expT"]
                w = (8 >> level) * QC
                nc.vector.tensor_add(e[:, :w], e[:, :w], e[:, w:2 * w])

            def emit_sum_recip(i):
                st = pend[i]
                pssum = pp.tile([128, QC], F32, tag="ps1", bufs=1,
                                name="pssum")
                nc.tensor.matmul(pssum[:1, :], ones_bf[:, :1],
                                 st["expT"][:, :QC], start=True, stop=True)
                rcp = atnp.tile([1, QC], F32, tag="rcp", bufs=1)
                nc.vector.reciprocal_approx_fast(rcp[:1, :], pssum[:1, :])
                rcpb = atnp.tile([1, QC], BF16, tag="rcpb", bufs=1)
                nc.vector.tensor_copy(rcpb[:1, :], rcp[:1, :])
                st["rcp"] = rcpb

            def emit_tail(i):
                """Broadcast 1/sum, normalize, scatter to a2a_in."""
                h, b, qc = CH[i]
                st = pend.pop(i)
                psb = pp.tile([128, QC], F32, tag="ps1", bufs=1, name="psb")
                nc.tensor.matmul(psb[:], ones_bf[:1, :], st["rcp"][:1, :],
                                 start=True, stop=True)
                rb = atnp.tile([128, QC], BF16, tag="rb", bufs=2)
                nc.vector.tensor_copy(rb[:], psb[:])
                att = atnp.tile([128, QC], BF16, tag="att", bufs=2)
                nc.vector.tensor_mul(att[:], st["pv"][:], rb[:])
                d0 = qc * 2
                nc.gpsimd.dma_start(a2a_in[h][b][d0, :, :], att[:, :RB])
                nc.gpsimd.dma_start(a2a_in[h][b][d0 + 1, :, :], att[:, RB:])
                if qc == 3:
                    emit_a2a(h, b)

            fillers = []

            def pull(rows_target):
                got = 0
                while got < rows_target and fillers:
                    try:
                        got += next(fillers[0])
                    except StopIteration:
                        fillers.pop(0)
                return got

            def drain_all():
                while fillers:
                    for _ in fillers.pop(0):
                        pass

            def attn_step(i, rows_budget):
                """Emit scores/exp of chunk i, PV + tree of i-1, tail of
                i-2."""
                cur = i < len(CH)
                prev = 0 <= i - 1 < len(CH)
                if i - 2 >= 0:
                    emit_tail(i - 2)
                if cur:
                    pend[i] = {"expT": atnp.tile([128, K16 * QC], BF16,
                                                 tag="expT", bufs=2,
                                                 name="expT")}
                per_k = max(rows_budget // K16, 384 if cur else 0)
                for k in range(K16):
                    if cur:
                        emit_sc_exp(i, k)
                    if prev:
                        emit_pv_k(i - 1, k)
                        if k == 9:
                            emit_tree(i - 1, 0)
                        elif k == 13:
                            emit_tree(i - 1, 1)
                        elif k == 15:
                            emit_tree(i - 1, 2)
                            emit_tree(i - 1, 3)
                    pull(per_k)
                if prev:
                    emit_sum_recip(i - 1)

            # ================= stage 1: QKV batch 0 =================
            with (
                tc.tile_pool(name="wqkv", bufs=1) as wp,
                tc.tile_pool(name="xwin", bufs=1) as xbp,
            ):
                def load_w(dram, tag):
                    dst = wp.tile([128, K16 * FL], BF16, tag=tag, name=tag)
                    for k in range(K16):
                        nc.sync.dma_start(dst[:, k * FL:(k + 1) * FL],
                                          dram.ap()[k * 128:(k + 1) * 128, :])
                    return dst

                def load_x(c):
                    dst = xbp.tile([128, K16 * CW], BF16, tag="xbf", bufs=2,
                                   name=f"xbf{c}")
                    for k in range(K16):
                        nc.sync.dma_start(
                            dst[:, k * CW:(k + 1) * CW],
                            xT_d.ap()[k * 128:(k + 1) * 128,
                                      c * CW:(c + 1) * CW])
                    xbf[c] = dst

                # interleave wq and x0 DMAs so the first q matmuls start
                # as soon as both streams land
                wq_sb = wp.tile([128, K16 * FL], BF16, tag="w_q", name="w_q")
                x0 = xbp.tile([128, K16 * CW], BF16, tag="xbf", bufs=2,
                              name="xbf0")
                for k in range(K16):
                    nc.sync.dma_start(wq_sb[:, k * FL:(k + 1) * FL],
                                      wqT_d.ap()[k * 128:(k + 1) * 128, :])
                    nc.sync.dma_start(x0[:, k * CW:(k + 1) * CW],
                                      xT_d.ap()[k * 128:(k + 1) * 128, 0:CW])
                xbf[0] = x0
                wk_sb = load_w(wkT_d, "w_k")
                wv_sb = load_w(wvT_d, "w_v")

                for c in range(4):
                    load_x(c + 1)
                    for _ in gen_qkv_qk(c):
                        pass
                    for _ in gen_qkv_v(c):
                        pass

                # ===== stage 2: attention b0 paced by QKV b1 =====
                def gen_qkv_b1():
                    for c in range(4, NCHUNK):
                        if c + 1 < NCHUNK:
                            load_x(c + 1)
                        yield from gen_qkv_qk(c)
                        yield from gen_qkv_v(c)

                fillers.append(gen_qkv_b1())
                for i in range(8):
                    attn_step(i, 26000 if i < 7 else 8000)
                # all b1 QKV must be emitted before b1 scores read it
                drain_all()

            # ===== stage 3: attention b1 paced by out-projection b0 =====
            with (
                tc.tile_pool(name="wo2", bufs=1) as wo2p,
                tc.tile_pool(name="aTp", bufs=1) as atp,
            ):
                wo2_sb = wo2p.tile([128, K16 * 1024], BF16)
                for k in range(K16):
                    nc.sync.dma_start(
                        wo2_sb[:, k * 1024:(k + 1) * 1024],
                        woT_d.ap()[k * 128:(k + 1) * 128, 1024:2048])
                aT0 = atp.tile([128, K16 * RB], BF16, name="aT0")
                aT1 = atp.tile([128, K16 * RB], BF16, name="aT1")

                def gen_proj_b0():
                    gather_aT(0, aT0, range(16))
                    yield 0
                    for m in range(2):
                        yield from gen_proj(0, m, 0, aT0, wo1_sb)

                for i in range(8, 16):
                    attn_step(i, 6500 if i >= 10 else 0)
                    if i == 9:
                        # a2a(1,0) was emitted in this step's tail
                        fillers.append(gen_proj_b0())
                    elif i == 13:
                        # a2a(0,1) landed; prefetch its half of aT1
                        gather_aT(1, aT1, range(0, 16, 2))
                # drain: chunk 14's tail, chunk 15 PV/tree/recip + tail
                attn_step(16, 6000)
                emit_tail(15)

                # ===== stage 4: reserved b0 pieces hide the last a2a =====
                gather_aT(1, aT1, range(1, 16, 2))
                drain_all()
                for m in range(2):
                    for _ in gen_proj(0, m, 1, aT0, wo2_sb):
                        pass
                for m in range(2):
                    for npair in range(2):
                        for _ in gen_proj(1, m, npair, aT1,
                                          wo1_sb if npair == 0 else wo2_sb):
                            pass

    nc.compile()
    return nc


def _get_nc():
    global _CACHED
    if _CACHED is None:
        _CACHED = _build()
    return _CACHED


def _prep_in_maps(x, Wq, bq, Wk, bk, Wv, bv, Wo, bo):
    import ml_dtypes

    BF = ml_dtypes.bfloat16
    xT = np.ascontiguousarray(x.reshape(BS, H).T.astype(BF))
    woT = np.ascontiguousarray(Wo.T.astype(BF))
    bo_bc = np.ascontiguousarray(np.broadcast_to(bo, (128, H)))
    ones_bf = np.ones((128, 128), BF)
    in_maps = []
    for c in range(NC):
        sl = slice(FL * c, FL * (c + 1))
        in_maps.append(
            {
                "xTb": xT,
                "wqTb": np.ascontiguousarray(Wq[sl, :].T.astype(BF)),
                "wkTb": np.ascontiguousarray(Wk[sl, :].T.astype(BF)),
                "wvTb": np.ascontiguousarray(Wv[sl, :].T.astype(BF)),
                "bq": np.ascontiguousarray(bq[sl].reshape(HL, 128).T),
                "bk": np.ascontiguousarray(bk[sl].reshape(HL, 128).T),
                "bv_bc2": np.ascontiguousarray(
                    np.broadcast_to(np.tile(bv[sl], 2), (128, 2 * FL))),
                "woTb": woT,
                "bo_bc": bo_bc,
                "ones_bf": ones_bf,
            }
        )
    return in_maps


def run(in_maps, trace=False):
    nc = _get_nc()
    return run_bass_kernel_spmd(nc, in_maps, core_ids=list(range(NC)), trace=trace)


def kernel(x, Wq, bq, Wk, bk, Wv, bv, Wo, bo):
    args = [np.asarray(a, dtype=np.float32)
            for a in (x, Wq, bq, Wk, bk, Wv, bv, Wo, bo)]
    in_maps = _prep_in_maps(*args)
    res = run(in_maps)
    out = np.empty((B, S, H), dtype=np.float32)
    for c in range(NC):
        oc = res.results[c]["out"]
        out[0, c * RB:(c + 1) * RB] = oc[:RB]
        out[1, c * RB:(c + 1) * RB] = oc[RB:]
    return out


# revision 17
# speedup vs baseline: 1.4671x; 1.0495x over previous
"""Multi-head self-attention (no mask) on 8 TRN2 NeuronCores.

Sharding: tensor-parallel over heads (2 heads/core) for QKV + attention,
per-(head,batch) AllToAlls re-shard to row-parallel for the output
projection.

v4: software-pipelined schedule that keeps the PE saturated end-to-end.
The PE row budget (~800k rows at the sustained-power clock) is the
floor; ACT (exp), DVE (softmax sums / normalize / biases) and the
collectives must all hide under it.  All f32->bf16 input casting is
done host-side (like the layout transposes), so DMA volume is halved
and ACT/DVE do no cast work.

  stage 1: QKV for batch 0.
  stage 2: attention(b0) chunk pipeline, paced by QKV(b1) filler.
  stage 3: attention(b1) pipeline, paced by out-projection(b0) filler.
  stage 4: reserved out-projection(b0) pieces hide the last AllToAll,
           then out-projection(b1).

Attention chunk pipeline (1 step per chunk, chunk state lags 1-2
steps): step i emits [scores_k(i) + exp_k(i) + PV_k(i-1) + filler]
per k so the PE stream never outruns ACT's exp throughput (718ns per
512-col tile vs 262ns per matmul).  Softmax denominators: in-place
pairwise DVE tree over expT (interleaved into the PV k-loop), PE
ones-matmul partition sum, 1-pass fast reciprocal, PE broadcast,
one DVE multiply.
"""

import numpy as np

import concourse.bass as bass
import concourse.tile as tile
from concourse import bacc, mybir
from concourse.bass_utils import run_bass_kernel_spmd

F32 = mybir.dt.float32
BF16 = mybir.dt.bfloat16

B, S, H = 2, 2048, 2048
NH, HD = 16, 128
NC = 8
BS = B * S          # 4096 tokens total
FL = H // NC        # 256 features per core (2 heads)
HL = NH // NC       # 2 heads per core
K16 = H // 128      # 16 contraction tiles
CW = 512            # QKV row-chunk width (tokens per chunk)
NCHUNK = BS // CW   # 8 chunks; 0-3 = batch 0, 4-7 = batch 1
QC = 512            # attention q-chunk width
RB = S // NC        # 256 rows per core per batch after a2a
SCALE = 1.0 / float(np.sqrt(HD))

_CACHED = None


def _build():
    nc = bacc.Bacc("TRN2", target_bir_lowering=False, debug=False, num_devices=NC)

    xT_d = nc.dram_tensor("xTb", [H, BS], BF16, kind="ExternalInput")
    wqT_d = nc.dram_tensor("wqTb", [H, FL], BF16, kind="ExternalInput")
    wkT_d = nc.dram_tensor("wkTb", [H, FL], BF16, kind="ExternalInput")
    wvT_d = nc.dram_tensor("wvTb", [H, FL], BF16, kind="ExternalInput")
    bq_d = nc.dram_tensor("bq", [128, HL], F32, kind="ExternalInput")
    bk_d = nc.dram_tensor("bk", [128, HL], F32, kind="ExternalInput")
    bv_d = nc.dram_tensor("bv_bc2", [128, 2 * FL], F32, kind="ExternalInput")
    woT_d = nc.dram_tensor("woTb", [H, H], BF16, kind="ExternalInput")
    bo_d = nc.dram_tensor("bo_bc", [128, H], F32, kind="ExternalInput")
    onesb_d = nc.dram_tensor("ones_bf", [128, 128], BF16, kind="ExternalInput")
    out_d = nc.dram_tensor("out", [2 * RB, H], F32, kind="ExternalOutput")

    with tile.TileContext(nc) as tc:
        with (
            tc.tile_pool(name="consts", bufs=1) as cstp,
            tc.tile_pool(name="dram", bufs=1, space="DRAM") as dp,
            tc.tile_pool(name="qkvout", bufs=1) as qkvp,
            tc.tile_pool(name="wo1", bufs=1) as wo1p,
            tc.tile_pool(name="attn", bufs=1) as atnp,
            tc.tile_pool(name="psum", bufs=1, space="PSUM") as pp,
        ):
            ones_bf = cstp.tile([128, 128], BF16)
            nc.sync.dma_start(ones_bf[:], onesb_d.ap()[:])
            bq_sb = cstp.tile([128, HL], F32)
            nc.sync.dma_start(bq_sb[:], bq_d.ap()[:])
            bk_sb = cstp.tile([128, HL], F32)
            nc.sync.dma_start(bk_sb[:], bk_d.ap()[:])
            bv_sb = cstp.tile([128, 2 * FL], F32)
            nc.sync.dma_start(bv_sb[:], bv_d.ap()[:])
            bo_sb = cstp.tile([128, H], F32)

            a2a_in = [[dp.tile([NC, 128, RB], BF16, name=f"a2ai{h}{b}")
                       for b in range(B)] for h in range(HL)]
            a2a_out = [[dp.tile([NC, 128, RB], BF16, name=f"a2ao{h}{b}")
                        for b in range(B)] for h in range(HL)]

            qT_sb = qkvp.tile([128, HL * BS], BF16)
            kT_sb = qkvp.tile([128, HL * BS], BF16)
            v_sb = qkvp.tile([128, (BS // 128) * FL], BF16)

            # Wo output-feature halves, k-major: wo1 = out cols 0:1024
            # (loaded during stage 1, after the startup-critical DMAs)
            wo1_sb = wo1p.tile([128, K16 * 1024], BF16)

            # ---------------- QKV emitters ----------------
            xbf = {}

            def gen_qkv_qk(c):
                """Generator: q/k matmuls of chunk c; yields rows per mm."""
                xc = xbf[c]
                for w_sb, b_sb, dst in ((wq_sb, bq_sb, qT_sb),
                                        (wk_sb, bk_sb, kT_sb)):
                    for m in range(HL):
                        ps = pp.tile([128, CW], F32, tag="qk", bufs=2)
                        for k in range(K16):
                            nc.tensor.matmul(
                                ps[:],
                                w_sb[:, k * FL + m * 128: k * FL + (m + 1) * 128],
                                xc[:, k * CW:(k + 1) * CW],
                                start=(k == 0), stop=(k == K16 - 1),
                            )
                            yield CW
                        nc.vector.tensor_scalar_add(
                            dst[:, m * BS + c * CW: m * BS + (c + 1) * CW],
                            ps[:], b_sb[:, m:m + 1],
                        )

            def gen_qkv_v(c):
                xc = xbf[c]
                for m2 in range(0, CW // 128, 2):
                    ps = pp.tile([128, 2 * FL], F32, tag="vv", bufs=1)
                    for half in range(2):
                        for k in range(K16):
                            nc.tensor.matmul(
                                ps[:, half * FL:(half + 1) * FL],
                                xc[:, k * CW + (m2 + half) * 128:
                                   k * CW + (m2 + half + 1) * 128],
                                wv_sb[:, k * FL:(k + 1) * FL],
                                start=(k == 0), stop=(k == K16 - 1),
                            )
                            yield FL
                    i = c * (CW // 128) + m2
                    nc.vector.tensor_add(
                        v_sb[:, i * FL:(i + 2) * FL], ps[:], bv_sb[:]
                    )

            # ---------------- out-projection emitters ----------------
            # even head-tiles first: they arrive one AllToAll earlier,
            # so the k-loop can start before the odd-head gather lands
            PROJ_K = list(range(0, K16, 2)) + list(range(1, K16, 2))

            def gen_proj(b, m, npair, aT, wo_sb):
                """rows [b*RB+m*128, +128), out cols npair*1024..+1024."""
                acc = [pp.tile([128, 512], F32, tag="qk", bufs=2,
                               name="acc") for _ in range(2)]
                for ki, k in enumerate(PROJ_K):
                    for j in range(2):
                        nc.tensor.matmul(
                            acc[j][:],
                            aT[:, k * RB + m * 128: k * RB + (m + 1) * 128],
                            wo_sb[:, k * 1024 + j * 512:
                                  k * 1024 + (j + 1) * 512],
                            start=(ki == 0), stop=(ki == K16 - 1),
                        )
                        yield 512
                for j in range(2):
                    n = npair * 2 + j
                    ot = atnp.tile([128, 512], F32, tag="ot", bufs=2)
                    nc.vector.tensor_add(
                        ot[:], acc[j][:], bo_sb[:, n * 512:(n + 1) * 512]
                    )
                    nc.sync.dma_start(
                        out_d.ap()[b * RB + m * 128: b * RB + (m + 1) * 128,
                                   n * 512:(n + 1) * 512],
                        ot[:],
                    )

            def gather_aT(b, aT, heads):
                for g in heads:
                    nc.sync.dma_start(
                        aT[:, g * RB:(g + 1) * RB],
                        a2a_out[g % 2][b][g // 2, :, :],
                    )

            def emit_a2a(h, b):
                nc.gpsimd.collective_compute(
                    "AllToAll",
                    mybir.AluOpType.bypass,
                    ins=[a2a_in[h][b].opt()],
                    outs=[a2a_out[h][b].opt()],
                    replica_groups=[list(range(NC))],
                )

            # ---------------- attention pipeline ----------------
            CH = [(h, b, qc) for b in range(B) for h in range(HL)
                  for qc in range(4)]
            pend = {}

            def emit_sc_exp(i, k):
                h, b, qc = CH[i]
                base = h * BS + b * S
                st = pend[i]
                sc = pp.tile([128, QC], F32, tag="sc", bufs=2, name="sc")
                nc.tensor.matmul(
                    sc[:],
                    kT_sb[:, base + k * 128: base + (k + 1) * 128],
                    qT_sb[:, base + qc * QC: base + (qc + 1) * QC],
                    start=True, stop=True,
                )
                nc.scalar.activation(
                    st["expT"][:, k * QC:(k + 1) * QC], sc[:],
                    mybir.ActivationFunctionType.Exp, scale=SCALE,
                )

            def emit_pv_k(i, k):
                h, b, qc = CH[i]
                st = pend[i]
                if k == 0:
                    st["pv"] = pp.tile([128, QC], F32, tag="pv", bufs=2,
                                       name="pv")
                nc.tensor.matmul(
                    st["pv"][:],
                    v_sb[:, (16 * b + k) * FL + h * 128:
                         (16 * b + k) * FL + (h + 1) * 128],
                    st["expT"][:, k * QC:(k + 1) * QC],
                    start=(k == 0), stop=(k == K16 - 1),
                )

            def emit_tree(i, level):
                """In-place pairwise tree level over expT (PV has already
                consumed the halves being overwritten)."""
                e = pend[i]["expT"]
                w = (8 >> level) * QC
                nc.vector.tensor_add(e[:, :w], e[:, :w], e[:, w:2 * w])

            def emit_sum_recip(i):
                st = pend[i]
                pssum = pp.tile([128, QC], F32, tag="ps1", bufs=1,
                                name="pssum")
                nc.tensor.matmul(pssum[:1, :], ones_bf[:, :1],
                                 st["expT"][:, :QC], start=True, stop=True)
                rcp = atnp.tile([1, QC], F32, tag="rcp", bufs=1)
                nc.vector.reciprocal_approx_fast(rcp[:1, :], pssum[:1, :])
                rcpb = atnp.tile([1, QC], BF16, tag="rcpb", bufs=1)
                nc.vector.tensor_copy(rcpb[:1, :], rcp[:1, :])
                st["rcp"] = rcpb

            def emit_tail(i):
                """Broadcast 1/sum, normalize, scatter to a2a_in."""
                h, b, qc = CH[i]
                st = pend.pop(i)
                psb = pp.tile([128, QC], F32, tag="ps1", bufs=1, name="psb")
                nc.tensor.matmul(psb[:], ones_bf[:1, :], st["rcp"][:1, :],
                                 start=True, stop=True)
                rb = atnp.tile([128, QC], BF16, tag="rb", bufs=2)
                nc.vector.tensor_copy(rb[:], psb[:])
                att = atnp.tile([128, QC], BF16, tag="att", bufs=2)
                nc.vector.tensor_mul(att[:], st["pv"][:], rb[:])
                d0 = qc * 2
                nc.gpsimd.dma_start(a2a_in[h][b][d0, :, :], att[:, :RB])
                nc.gpsimd.dma_start(a2a_in[h][b][d0 + 1, :, :], att[:, RB:])
                if qc == 3:
                    emit_a2a(h, b)

            fillers = []

            def pull(rows_target):
                got = 0
                while got < rows_target and fillers:
                    try:
                        got += next(fillers[0])
                    except StopIteration:
                        fillers.pop(0)
                return got

            def drain_all():
                while fillers:
                    for _ in fillers.pop(0):
                        pass

            def attn_step(i, rows_budget):
                """Emit scores/exp of chunk i, PV + tree of i-1, tail of
                i-2."""
                cur = i < len(CH)
                prev = 0 <= i - 1 < len(CH)
                if i - 2 >= 0:
                    emit_tail(i - 2)
                if cur:
                    pend[i] = {"expT": atnp.tile([128, K16 * QC], BF16,
                                                 tag="expT", bufs=2,
                                                 name="expT")}
                per_k = max(rows_budget // K16, 384 if cur else 0)
                for k in range(K16):
                    if cur:
                        emit_sc_exp(i, k)
                    if prev:
                        emit_pv_k(i - 1, k)
                        if k == 9:
                            emit_tree(i - 1, 0)
                        elif k == 13:
                            emit_tree(i - 1, 1)
                        elif k == 15:
                            emit_tree(i - 1, 2)
                            emit_tree(i - 1, 3)
                    pull(per_k)
                if prev:
                    emit_sum_recip(i - 1)

            # ================= stage 1: QKV batch 0 =================
            with (
                tc.tile_pool(name="wqkv", bufs=1) as wp,
                tc.tile_pool(name="xwin", bufs=1) as xbp,
            ):
                def load_w(dram, tag):
                    dst = wp.tile([128, K16 * FL], BF16, tag=tag, name=tag)
                    for k in range(K16):
                        nc.sync.dma_start(dst[:, k * FL:(k + 1) * FL],
                                          dram.ap()[k * 128:(k + 1) * 128, :])
                    return dst

                def load_x(c):
                    dst = xbp.tile([128, K16 * CW], BF16, tag="xbf", bufs=2,
                                   name=f"xbf{c}")
                    for k in range(K16):
                        nc.sync.dma_start(
                            dst[:, k * CW:(k + 1) * CW],
                            xT_d.ap()[k * 128:(k + 1) * 128,
                                      c * CW:(c + 1) * CW])
                    xbf[c] = dst

                # interleave wq and x0 DMAs so the first q matmuls start
                # as soon as both streams land
                wq_sb = wp.tile([128, K16 * FL], BF16, tag="w_q", name="w_q")
                x0 = xbp.tile([128, K16 * CW], BF16, tag="xbf", bufs=2,
                              name="xbf0")
                for k in range(K16):
                    nc.sync.dma_start(wq_sb[:, k * FL:(k + 1) * FL],
                                      wqT_d.ap()[k * 128:(k + 1) * 128, :])
                    nc.sync.dma_start(x0[:, k * CW:(k + 1) * CW],
                                      xT_d.ap()[k * 128:(k + 1) * 128, 0:CW])
                xbf[0] = x0
                wk_sb = load_w(wkT_d, "w_k")
                wv_sb = load_w(wvT_d, "w_v")

                for c in range(4):
                    load_x(c + 1)
                    for _ in gen_qkv_qk(c):
                        pass
                    for _ in gen_qkv_v(c):
                        pass
                    # wo1 rides the DMA slack behind the x loads
                    for k in range(4 * c, 4 * (c + 1)):
                        nc.sync.dma_start(
                            wo1_sb[:, k * 1024:(k + 1) * 1024],
                            woT_d.ap()[k * 128:(k + 1) * 128, 0:1024])
                nc.sync.dma_start(bo_sb[:], bo_d.ap()[:])

                # ===== stage 2: attention b0 paced by QKV b1 =====
                def gen_qkv_b1():
                    for c in range(4, NCHUNK):
                        if c + 1 < NCHUNK:
                            load_x(c + 1)
                        yield from gen_qkv_qk(c)
                        yield from gen_qkv_v(c)

                fillers.append(gen_qkv_b1())
                for i in range(8):
                    attn_step(i, 26000 if i < 7 else 8000)
                # all b1 QKV must be emitted before b1 scores read it
                drain_all()

            # ===== stage 3: attention b1 paced by out-projection b0 =====
            with (
                tc.tile_pool(name="wo2", bufs=1) as wo2p,
                tc.tile_pool(name="aTp", bufs=1) as atp,
            ):
                wo2_sb = wo2p.tile([128, K16 * 1024], BF16)
                aT0 = atp.tile([128, K16 * RB], BF16, name="aT0")
                aT1 = atp.tile([128, K16 * RB], BF16, name="aT1")

                def gen_proj_b0():
                    for m in range(2):
                        yield from gen_proj(0, m, 0, aT0, wo1_sb)

                for i in range(8, 16):
                    attn_step(i, 6000 if i >= 10 else 0)
                    if i == 8:
                        # a2a(0,0) is long done: its head-tiles of aT0
                        gather_aT(0, aT0, range(0, 16, 2))
                    elif i == 10:
                        # a2a(1,0) was emitted in step 9's tail
                        gather_aT(0, aT0, range(1, 16, 2))
                        fillers.append(gen_proj_b0())
                    elif i == 12:
                        # wo2 loads held back so they don't contend with
                        # the a2a(1,0) transport on the DMA engines
                        for k in range(K16):
                            nc.sync.dma_start(
                                wo2_sb[:, k * 1024:(k + 1) * 1024],
                                woT_d.ap()[k * 128:(k + 1) * 128, 1024:2048])
                    elif i == 13:
                        # a2a(0,1) landed; prefetch its half of aT1
                        gather_aT(1, aT1, range(0, 16, 2))
                # drain: chunk 14's tail, chunk 15 PV/tree/recip + tail
                attn_step(16, 6000)
                emit_tail(15)

                # ===== stage 4: reserved b0 pieces hide the last a2a =====
                gather_aT(1, aT1, range(1, 16, 2))
                drain_all()
                for m in range(2):
                    for _ in gen_proj(0, m, 1, aT0, wo2_sb):
                        pass
                for m in range(2):
                    for npair in range(2):
                        for _ in gen_proj(1, m, npair, aT1,
                                          wo1_sb if npair == 0 else wo2_sb):
                            pass

    nc.compile()
    return nc


def _get_nc():
    global _CACHED
    if _CACHED is None:
        _CACHED = _build()
    return _CACHED


def _prep_in_maps(x, Wq, bq, Wk, bk, Wv, bv, Wo, bo):
    import ml_dtypes

    BF = ml_dtypes.bfloat16
    xT = np.ascontiguousarray(x.reshape(BS, H).T.astype(BF))
    woT = np.ascontiguousarray(Wo.T.astype(BF))
    bo_bc = np.ascontiguousarray(np.broadcast_to(bo, (128, H)))
    ones_bf = np.ones((128, 128), BF)
    in_maps = []
    for c in range(NC):
        sl = slice(FL * c, FL * (c + 1))
        in_maps.append(
            {
                "xTb": xT,
                "wqTb": np.ascontiguousarray(Wq[sl, :].T.astype(BF)),
                "wkTb": np.ascontiguousarray(Wk[sl, :].T.astype(BF)),
                "wvTb": np.ascontiguousarray(Wv[sl, :].T.astype(BF)),
                "bq": np.ascontiguousarray(bq[sl].reshape(HL, 128).T),
                "bk": np.ascontiguousarray(bk[sl].reshape(HL, 128).T),
                "bv_bc2": np.ascontiguousarray(
                    np.broadcast_to(np.tile(bv[sl], 2), (128, 2 * FL))),
                "woTb": woT,
                "bo_bc": bo_bc,
                "ones_bf": ones_bf,
            }
        )
    return in_maps


def run(in_maps, trace=False):
    nc = _get_nc()
    return run_bass_kernel_spmd(nc, in_maps, core_ids=list(range(NC)), trace=trace)


def kernel(x, Wq, bq, Wk, bk, Wv, bv, Wo, bo):
    args = [np.asarray(a, dtype=np.float32)
            for a in (x, Wq, bq, Wk, bk, Wv, bv, Wo, bo)]
    in_maps = _prep_in_maps(*args)
    res = run(in_maps)
    out = np.empty((B, S, H), dtype=np.float32)
    for c in range(NC):
        oc = res.results[c]["out"]
        out[0, c * RB:(c + 1) * RB] = oc[:RB]
        out[1, c * RB:(c + 1) * RB] = oc[RB:]
    return out
